# revision 7
# baseline (speedup 1.0000x reference)
"""MultiHeadLiftLayer Trainium2 kernel.

reference:
    edge_signal = relu(x_0[src] @ W[:C] + x_0[tgt] @ W[C:])   # [E, 8]
    out = concat([edge_signal, x_1], axis=1)                   # [E, 72]

Strategy (8 NeuronCores, edges sharded):
  - Precompute per-node projections P_src = x_0 @ W[:C], P_tgt = x_0 @ W[C:]
    (each [N, 8]) on the tensor engine, stored as an f16 pair-packed table in
    SBUF: partition p holds one head-column (heads replicated; partitions
    0-63 = src heads, 64-127 = tgt heads), two consecutive nodes packed per
    u32 element -> num_elems 25000 fits ap_gather's int16-delta constraint.
  - Per 8192-edge call: GPSIMD ap_gather fetches the node pair for each
    edge (groups 0-3 use src indices of chunks 0-3, groups 4-7 tgt indices),
    DVE selects the even/odd f16 by node parity (host-provided u8 mask), and
    one PE matmul per 128-edge block against a fixed 0/1 selector sums the
    src/tgt lanes per head while landing directly in [edge, head] PSUM
    orientation. Rows are assembled in SBUF (p-major: partition p owns 64
    consecutive edges, so x_1 loads and output stores are one contiguous
    16-18KB DRAM run per partition) and stored with relu fused into the
    PSUM->SBUF copies.

    Measured on trn2: ap_gather runs ~28ns/idx (SBUF round-trip bound in the
    ucode, 4 idx per pipelined-depth-1 request); with 2 idx/edge spread over
    8 Q7 cores that is ~7ns/edge = ~545us for 78125 edges/core, which bounds
    the kernel; all DMA/PE/DVE/ACT work hides underneath it.
"""
import sys

sys.path.insert(0, "/opt/trn_rl_repo")

import numpy as np
import concourse.bass as bass
import concourse.tile as tile
from concourse import bacc, mybir
from concourse.bass_utils import run_bass_kernel_spmd

NUM_NODES = 50000
IN_CH0 = 128
HEADS = 8
NUM_EDGES = 625000
IN_CH1 = 64
OUT_CH = HEADS + IN_CH1  # 72

N_CORES = 8
E_CORE = NUM_EDGES // N_CORES  # 78125
L_MAIN = 2048                  # gather indices per call (per 16-partition group)
N_MAIN = 9                     # main calls: 9 * 4 * 2048 = 73728 edges
L_TAIL = 1152                  # tail call: 4 * 1152 = 4608 slots, 4397 valid
CALL_LS = [L_MAIN] * N_MAIN + [L_TAIL]
NPAIR = NUM_NODES // 2         # 25000 u32 elements per table column
NT = 2000                      # node-tile for the projection matmul
PCHUNK = 500                   # psum free-dim chunk

_cache = {}


def _build_program():
    if "nc" in _cache:
        return _cache["nc"]
    nc = bacc.Bacc("TRN2", target_bir_lowering=False, debug=False,
                   num_devices=N_CORES)
    f32, f16, i16 = mybir.dt.float32, mybir.dt.float16, mybir.dt.int16
    u8 = mybir.dt.uint8

    x0t = nc.dram_tensor("x0t", [IN_CH0, NUM_NODES], f16, kind="ExternalInput").ap()
    wbig = nc.dram_tensor("wbig", [IN_CH0, 128], f32, kind="ExternalInput").ap()
    x1 = nc.dram_tensor("x1", [E_CORE, IN_CH1], f32, kind="ExternalInput").ap()
    idx_in = nc.dram_tensor("idx", [len(CALL_LS), 128, L_MAIN // 16], i16,
                            kind="ExternalInput").ap()
    msel_in = nc.dram_tensor("msel", [128, 32], f32, kind="ExternalInput").ap()
    mask_in = nc.dram_tensor("mask", [len(CALL_LS), 128, L_MAIN], u8,
                             kind="ExternalInput").ap()
    out = nc.dram_tensor("out", [E_CORE, OUT_CH], f32, kind="ExternalOutput").ap()

    with tile.TileContext(nc) as tc:
        with tc.tile_pool(name="tab", bufs=1) as tab_pool, \
             tc.tile_pool(name="const", bufs=1) as const_pool:
            tab = tab_pool.tile([128, NPAIR], f32)       # f16 pair-packed view
            tab_f16 = tab[:].bitcast(f16)                # [128, 50000]
            msel32 = const_pool.tile([128, 32], f32)
            nc.sync.dma_start(msel32[:], msel_in[:])
            msel = const_pool.tile([128, 32], f16)
            nc.vector.tensor_copy(msel[:], msel32[:])

            # ---- phase 1: build the projection table ----
            with tc.tile_pool(name="p1", bufs=4) as p1_pool, \
                 tc.tile_pool(name="p1w", bufs=1) as p1w_pool, \
                 tc.tile_pool(name="p1ps", bufs=6, space="PSUM") as p1ps:
                wb32 = p1w_pool.tile([128, 128], f32)
                nc.sync.dma_start(wb32[:], wbig[:])
                wb16 = p1w_pool.tile([128, 128], f16)
                nc.vector.tensor_copy(wb16[:], wb32[:])
                for t in range(NUM_NODES // NT):
                    xt = p1_pool.tile([128, NT], f16, tag="xt")
                    nc.sync.dma_start(xt[:], x0t[:, t * NT:(t + 1) * NT])
                    for c in range(NT // PCHUNK):
                        ps = p1ps.tile([128, PCHUNK], f32)
                        nc.tensor.matmul(ps[:], lhsT=wb16[:],
                                         rhs=xt[:, c * PCHUNK:(c + 1) * PCHUNK],
                                         start=True, stop=True)
                        n0 = t * NT + c * PCHUNK
                        dst = tab_f16[:, n0:n0 + PCHUNK]
                        if c % 2 == 0:
                            nc.vector.tensor_copy(dst, ps[:])
                        else:
                            nc.scalar.copy(dst, ps[:])

            # ---- phase 2: gather / combine / emit ----
            with tc.tile_pool(name="io", bufs=3) as io_pool, \
                 tc.tile_pool(name="idxp", bufs=1) as idx_pool, \
                 tc.tile_pool(name="mega", bufs=2) as mega_pool, \
                 tc.tile_pool(name="p2ps", bufs=2, space="PSUM") as p2ps:
                its = []
                for k, L in enumerate(CALL_LS):
                    it = idx_pool.tile([128, L_MAIN // 16], i16, tag=f"it{k}")
                    nc.sync.dma_start(it[:, :L // 16], idx_in[k, :, :L // 16])
                    its.append(it)
                e_base = 0
                for k, L in enumerate(CALL_LS):
                    nseg = 4 * L // 128
                    it = its[k]
                    mk = io_pool.tile([128, L_MAIN], u8, tag="mk")
                    nc.sync.dma_start(mk[:, :L], mask_in[k, :, :L])

                    ot = io_pool.tile([128, L_MAIN], f32, tag="ot")
                    nc.gpsimd.ap_gather(out_ap=ot[:, :L], in_ap=tab[:],
                                        idxs_ap=it[:, :L // 16], channels=128,
                                        num_elems=NPAIR, d=1, num_idxs=L)
                    pair = ot[:, :L].bitcast(f16).rearrange(
                        "p (l two) -> p l two", two=2)
                    sel = io_pool.tile([128, L_MAIN], f16, tag="sel")
                    nc.vector.tensor_copy(sel[:, :L], pair[:, :, 0])
                    nc.vector.copy_predicated(sel[:, :L], mk[:, :L], pair[:, :, 1])

                    # per 128-edge block: one PE matmul sums the src lane
                    # and tgt lane per head (fixed 0/1 selector as the moving
                    # operand) and lands directly in [edge, head] orientation:
                    # psum[e, 8g+h] = sel[16g+h, e] + sel[64+16g+h, e]
                    nb = L // 128
                    ps2 = p2ps.tile([128, 512], f32)
                    for b in range(nb):
                        nc.tensor.matmul(ps2[:, 32 * b:32 * b + 32],
                                         lhsT=sel[:, 128 * b:128 * (b + 1)],
                                         rhs=msel[:], start=True, stop=True)

                    mega = mega_pool.tile([128, 64, OUT_CH], f32)
                    # relu fused into the PSUM->SBUF copies; chunk g block b
                    # sits at psum cols [32b + 8g : +8], destination seg g*nb+b
                    psv = ps2[:, :32 * nb].rearrange("p (s h) -> p s h", h=32)
                    for g in range(4):
                        nc.scalar.activation(
                            mega[:, g * nb:(g + 1) * nb, :HEADS],
                            psv[:, :, 8 * g:8 * g + 8],
                            mybir.ActivationFunctionType.Relu)

                    if k < N_MAIN:
                        # p-major: partition p holds edges [e_base+64p, +64),
                        # giving one contiguous 16-18KB DRAM run per partition
                        v = slice(e_base, e_base + 4 * L)
                        nc.sync.dma_start(
                            mega[:, :, HEADS:],
                            x1[v].rearrange("(p s) c -> p s c", s=64))
                        nc.scalar.dma_start(
                            out[v].rearrange("(p s) c -> p s c", s=64),
                            mega[:])
                    else:
                        # tail: seg-major with partial coverage
                        n_edges = min(E_CORE - e_base, 4 * L)
                        full_seg = n_edges // 128
                        rem = n_edges - full_seg * 128
                        if full_seg:
                            v = slice(e_base, e_base + full_seg * 128)
                            nc.sync.dma_start(
                                mega[:, :full_seg, HEADS:],
                                x1[v].rearrange("(s p) c -> p s c", p=128))
                            nc.scalar.dma_start(
                                out[v].rearrange("(s p) c -> p s c", p=128),
                                mega[:, :full_seg, :])
                        if rem:
                            v = slice(e_base + full_seg * 128, e_base + n_edges)
                            nc.sync.dma_start(mega[:rem, full_seg, HEADS:], x1[v])
                            nc.scalar.dma_start(out[v], mega[:rem, full_seg, :])
                    e_base += 4 * L

    nc.compile()
    _cache["nc"] = nc
    return nc


def _prep_inputs(x_0, adjacency_0, x_1, att_parameter):
    x0t = np.ascontiguousarray(np.asarray(x_0).T).astype(np.float16)
    wbig = np.empty((IN_CH0, 128), np.float32)
    for p in range(128):
        half = IN_CH0 * (p >= 64)
        wbig[:, p] = att_parameter[half:half + IN_CH0, p % 8]

    msel = np.zeros((128, 32), np.float32)
    for g in range(4):
        for h in range(8):
            msel[16 * g + h, 8 * g + h] = 1.0
            msel[64 + 16 * g + h, 8 * g + h] = 1.0

    src_all = np.asarray(adjacency_0[0]).astype(np.int64)
    tgt_all = np.asarray(adjacency_0[1]).astype(np.int64)
    x_1 = np.asarray(x_1, dtype=np.float32)

    in_maps = []
    for core in range(N_CORES):
        lo = core * E_CORE
        src = src_all[lo:lo + E_CORE]
        tgt = tgt_all[lo:lo + E_CORE]
        idx_a = np.zeros((len(CALL_LS), 128, L_MAIN // 16), np.int16)
        mask_a = np.zeros((len(CALL_LS), 128, L_MAIN), np.uint8)
        e = 0
        pos = np.arange(L_MAIN)
        pmaj = 64 * (pos % 128) + (pos // 128)  # i = 128b+p -> 64p + b
        for k, L in enumerate(CALL_LS):
            for g in range(4):
                if k < N_MAIN:
                    eoff = e + pmaj + 16 * g
                    sv = src[eoff]
                    tv = tgt[eoff]
                else:
                    c0 = e + g * L
                    sv = src[c0:c0 + L]
                    tv = tgt[c0:c0 + L]
                    if len(sv) < L:  # tail padding
                        sv = np.concatenate([sv, np.zeros(L - len(sv), np.int64)])
                        tv = np.concatenate([tv, np.zeros(L - len(tv), np.int64)])
                # wrapped: idxs[p, s] = v[16 s + p]
                idx_a[k, 16 * g:16 * g + 16, :L // 16] = \
                    (sv >> 1).astype(np.int16).reshape(L // 16, 16).T
                idx_a[k, 64 + 16 * g:64 + 16 * g + 16, :L // 16] = \
                    (tv >> 1).astype(np.int16).reshape(L // 16, 16).T
                mask_a[k, 16 * g:16 * g + 16, :L] = \
                    (sv & 1).astype(np.uint8)[None, :]
                mask_a[k, 64 + 16 * g:64 + 16 * g + 16, :L] = \
                    (tv & 1).astype(np.uint8)[None, :]
            e += 4 * L
        in_maps.append({
            "x0t": x0t,
            "wbig": wbig,
            "msel": msel,
            "x1": x_1[lo:lo + E_CORE],
            "idx": idx_a,
            "mask": mask_a,
        })
    return in_maps


def kernel(x_0, adjacency_0, x_1, att_parameter, _trace=False):
    # materialize as numpy up front: slicing jax arrays here would trigger
    # device jit compiles of generic XLA ops, which this toolchain rejects
    x_0 = np.asarray(x_0, dtype=np.float32)
    adjacency_0 = np.asarray(adjacency_0)
    x_1 = np.asarray(x_1, dtype=np.float32)
    att_parameter = np.asarray(att_parameter, dtype=np.float32)
    nc = _build_program()
    in_maps = _prep_inputs(x_0, adjacency_0, x_1, att_parameter)
    res = run_bass_kernel_spmd(nc, in_maps, list(range(N_CORES)), trace=_trace)
    out = np.concatenate([res.results[i]["out"] for i in range(N_CORES)], axis=0)
    kernel.last_exec_time_ns = res.exec_time_ns
    return out


# revision 9
# speedup vs baseline: 1.0004x; 1.0004x over previous
"""MultiHeadLiftLayer Trainium2 kernel.

reference:
    edge_signal = relu(x_0[src] @ W[:C] + x_0[tgt] @ W[C:])   # [E, 8]
    out = concat([edge_signal, x_1], axis=1)                   # [E, 72]

Strategy (8 NeuronCores, edges sharded):
  - Precompute per-node projections P_src = x_0 @ W[:C], P_tgt = x_0 @ W[C:]
    (each [N, 8]) on the tensor engine, stored as an f16 pair-packed table in
    SBUF: partition p holds one head-column (heads replicated; partitions
    0-63 = src heads, 64-127 = tgt heads), two consecutive nodes packed per
    u32 element -> num_elems 25000 fits ap_gather's int16-delta constraint.
  - Per 8192-edge call: GPSIMD ap_gather fetches the node pair for each
    edge (groups 0-3 use src indices of chunks 0-3, groups 4-7 tgt indices),
    DVE selects the even/odd f16 by node parity (host-provided u8 mask), and
    one PE matmul per 128-edge block against a fixed 0/1 selector sums the
    src/tgt lanes per head while landing directly in [edge, head] PSUM
    orientation. Rows are assembled in SBUF (p-major: partition p owns 64
    consecutive edges, so x_1 loads and output stores are one contiguous
    16-18KB DRAM run per partition) and stored with relu fused into the
    PSUM->SBUF copies.

    Measured on trn2: ap_gather runs ~28ns/idx (SBUF round-trip bound in the
    ucode, 4 idx per pipelined-depth-1 request); with 2 idx/edge spread over
    8 Q7 cores that is ~7ns/edge = ~545us for 78125 edges/core, which bounds
    the kernel; all DMA/PE/DVE/ACT work hides underneath it.
"""
import sys

sys.path.insert(0, "/opt/trn_rl_repo")

import numpy as np
import concourse.bass as bass
import concourse.tile as tile
from concourse import bacc, mybir
from concourse.bass_utils import run_bass_kernel_spmd

NUM_NODES = 50000
IN_CH0 = 128
HEADS = 8
NUM_EDGES = 625000
IN_CH1 = 64
OUT_CH = HEADS + IN_CH1  # 72

N_CORES = 8
E_CORE = NUM_EDGES // N_CORES  # 78125
L_MAIN = 2048                  # gather indices per call (per 16-partition group)
N_MAIN = 9                     # main calls: 9 * 4 * 2048 = 73728 edges
L_TAIL = 1152                  # tail call: 4 * 1152 = 4608 slots, 4397 valid
CALL_LS = [L_MAIN] * N_MAIN + [L_TAIL]
NPAIR = NUM_NODES // 2         # 25000 u32 elements per table column
NT = 2000                      # node-tile for the projection matmul
PCHUNK = 500                   # psum free-dim chunk

_cache = {}


def _build_program():
    if "nc" in _cache:
        return _cache["nc"]
    nc = bacc.Bacc("TRN2", target_bir_lowering=False, debug=False,
                   num_devices=N_CORES)
    f32, f16, i16 = mybir.dt.float32, mybir.dt.float16, mybir.dt.int16
    u8 = mybir.dt.uint8

    x0t = nc.dram_tensor("x0t", [IN_CH0, NUM_NODES], f16, kind="ExternalInput").ap()
    wbig = nc.dram_tensor("wbig", [IN_CH0, 128], f32, kind="ExternalInput").ap()
    x1 = nc.dram_tensor("x1", [E_CORE, IN_CH1], f32, kind="ExternalInput").ap()
    idx_in = nc.dram_tensor("idx", [len(CALL_LS), 128, L_MAIN // 16], i16,
                            kind="ExternalInput").ap()
    msel_in = nc.dram_tensor("msel", [128, 32], f32, kind="ExternalInput").ap()
    mask_in = nc.dram_tensor("mask", [len(CALL_LS), 128, L_MAIN], u8,
                             kind="ExternalInput").ap()
    out = nc.dram_tensor("out", [E_CORE, OUT_CH], f32, kind="ExternalOutput").ap()

    with tile.TileContext(nc) as tc:
        with tc.tile_pool(name="tab", bufs=1) as tab_pool, \
             tc.tile_pool(name="const", bufs=1) as const_pool:
            tab = tab_pool.tile([128, NPAIR], f32)       # f16 pair-packed view
            tab_f16 = tab[:].bitcast(f16)                # [128, 50000]
            msel32 = const_pool.tile([128, 32], f32)
            nc.sync.dma_start(msel32[:], msel_in[:])
            msel = const_pool.tile([128, 32], f16)
            nc.vector.tensor_copy(msel[:], msel32[:])

            # ---- phase 1: build the projection table ----
            with tc.tile_pool(name="p1", bufs=5) as p1_pool, \
                 tc.tile_pool(name="p1w", bufs=1) as p1w_pool, \
                 tc.tile_pool(name="p1ps", bufs=8, space="PSUM") as p1ps:
                wb32 = p1w_pool.tile([128, 128], f32)
                nc.sync.dma_start(wb32[:], wbig[:])
                wb16 = p1w_pool.tile([128, 128], f16)
                nc.vector.tensor_copy(wb16[:], wb32[:])
                ci = 0
                for t in range(NUM_NODES // NT):
                    xt = p1_pool.tile([128, NT], f16, tag="xt")
                    nc.sync.dma_start(xt[:], x0t[:, t * NT:(t + 1) * NT])
                    for c in range(NT // PCHUNK):
                        ps = p1ps.tile([128, PCHUNK], f32)
                        nc.tensor.matmul(ps[:], lhsT=wb16[:],
                                         rhs=xt[:, c * PCHUNK:(c + 1) * PCHUNK],
                                         start=True, stop=True)
                        n0 = t * NT + c * PCHUNK
                        dst = tab_f16[:, n0:n0 + PCHUNK]
                        if ci % 2 == 0:
                            nc.vector.tensor_copy(dst, ps[:])
                        else:
                            nc.scalar.copy(dst, ps[:])
                        ci += 1

            # ---- phase 2: gather / combine / emit ----
            with tc.tile_pool(name="io", bufs=3) as io_pool, \
                 tc.tile_pool(name="idxp", bufs=1) as idx_pool, \
                 tc.tile_pool(name="mega", bufs=2) as mega_pool, \
                 tc.tile_pool(name="p2ps", bufs=2, space="PSUM") as p2ps:
                its = []
                for k, L in enumerate(CALL_LS):
                    it = idx_pool.tile([128, L_MAIN // 16], i16, tag=f"it{k}")
                    nc.sync.dma_start(it[:, :L // 16], idx_in[k, :, :L // 16])
                    its.append(it)
                e_base = 0
                for k, L in enumerate(CALL_LS):
                    nseg = 4 * L // 128
                    it = its[k]
                    mk = io_pool.tile([128, L_MAIN], u8, tag="mk")
                    nc.sync.dma_start(mk[:, :L], mask_in[k, :, :L])

                    ot = io_pool.tile([128, L_MAIN], f32, tag="ot")
                    nc.gpsimd.ap_gather(out_ap=ot[:, :L], in_ap=tab[:],
                                        idxs_ap=it[:, :L // 16], channels=128,
                                        num_elems=NPAIR, d=1, num_idxs=L)
                    pair = ot[:, :L].bitcast(f16).rearrange(
                        "p (l two) -> p l two", two=2)
                    sel = io_pool.tile([128, L_MAIN], f16, tag="sel")
                    nc.vector.tensor_copy(sel[:, :L], pair[:, :, 0])
                    nc.vector.copy_predicated(sel[:, :L], mk[:, :L], pair[:, :, 1])

                    # per 128-edge block: one PE matmul sums the src lane
                    # and tgt lane per head (fixed 0/1 selector as the moving
                    # operand) and lands directly in [edge, head] orientation:
                    # psum[e, 8g+h] = sel[16g+h, e] + sel[64+16g+h, e]
                    nb = L // 128
                    ps2 = p2ps.tile([128, 512], f32)
                    for b in range(nb):
                        nc.tensor.matmul(ps2[:, 32 * b:32 * b + 32],
                                         lhsT=sel[:, 128 * b:128 * (b + 1)],
                                         rhs=msel[:], start=True, stop=True)

                    mega = mega_pool.tile([128, 64, OUT_CH], f32)
                    # relu fused into the PSUM->SBUF copies; chunk g block b
                    # sits at psum cols [32b + 8g : +8], destination seg g*nb+b
                    psv = ps2[:, :32 * nb].rearrange("p (s h) -> p s h", h=32)
                    for g in range(4):
                        nc.scalar.activation(
                            mega[:, g * nb:(g + 1) * nb, :HEADS],
                            psv[:, :, 8 * g:8 * g + 8],
                            mybir.ActivationFunctionType.Relu)

                    if k < N_MAIN:
                        # p-major: partition p holds edges [e_base+64p, +64),
                        # giving one contiguous 16-18KB DRAM run per partition
                        v = slice(e_base, e_base + 4 * L)
                        nc.sync.dma_start(
                            mega[:, :, HEADS:],
                            x1[v].rearrange("(p s) c -> p s c", s=64))
                        nc.scalar.dma_start(
                            out[v].rearrange("(p s) c -> p s c", s=64),
                            mega[:])
                    else:
                        # tail: seg-major with partial coverage
                        n_edges = min(E_CORE - e_base, 4 * L)
                        full_seg = n_edges // 128
                        rem = n_edges - full_seg * 128
                        if full_seg:
                            v = slice(e_base, e_base + full_seg * 128)
                            nc.sync.dma_start(
                                mega[:, :full_seg, HEADS:],
                                x1[v].rearrange("(s p) c -> p s c", p=128))
                            nc.scalar.dma_start(
                                out[v].rearrange("(s p) c -> p s c", p=128),
                                mega[:, :full_seg, :])
                        if rem:
                            v = slice(e_base + full_seg * 128, e_base + n_edges)
                            nc.sync.dma_start(mega[:rem, full_seg, HEADS:], x1[v])
                            nc.scalar.dma_start(out[v], mega[:rem, full_seg, :])
                    e_base += 4 * L

    nc.compile()
    _cache["nc"] = nc
    return nc


def _prep_inputs(x_0, adjacency_0, x_1, att_parameter):
    x0t = np.ascontiguousarray(np.asarray(x_0).T).astype(np.float16)
    wbig = np.empty((IN_CH0, 128), np.float32)
    for p in range(128):
        half = IN_CH0 * (p >= 64)
        wbig[:, p] = att_parameter[half:half + IN_CH0, p % 8]

    msel = np.zeros((128, 32), np.float32)
    for g in range(4):
        for h in range(8):
            msel[16 * g + h, 8 * g + h] = 1.0
            msel[64 + 16 * g + h, 8 * g + h] = 1.0

    src_all = np.asarray(adjacency_0[0]).astype(np.int64)
    tgt_all = np.asarray(adjacency_0[1]).astype(np.int64)
    x_1 = np.asarray(x_1, dtype=np.float32)

    in_maps = []
    for core in range(N_CORES):
        lo = core * E_CORE
        src = src_all[lo:lo + E_CORE]
        tgt = tgt_all[lo:lo + E_CORE]
        idx_a = np.zeros((len(CALL_LS), 128, L_MAIN // 16), np.int16)
        mask_a = np.zeros((len(CALL_LS), 128, L_MAIN), np.uint8)
        e = 0
        pos = np.arange(L_MAIN)
        pmaj = 64 * (pos % 128) + (pos // 128)  # i = 128b+p -> 64p + b
        for k, L in enumerate(CALL_LS):
            for g in range(4):
                if k < N_MAIN:
                    eoff = e + pmaj + 16 * g
                    sv = src[eoff]
                    tv = tgt[eoff]
                else:
                    c0 = e + g * L
                    sv = src[c0:c0 + L]
                    tv = tgt[c0:c0 + L]
                    if len(sv) < L:  # tail padding
                        sv = np.concatenate([sv, np.zeros(L - len(sv), np.int64)])
                        tv = np.concatenate([tv, np.zeros(L - len(tv), np.int64)])
                # wrapped: idxs[p, s] = v[16 s + p]
                idx_a[k, 16 * g:16 * g + 16, :L // 16] = \
                    (sv >> 1).astype(np.int16).reshape(L // 16, 16).T
                idx_a[k, 64 + 16 * g:64 + 16 * g + 16, :L // 16] = \
                    (tv >> 1).astype(np.int16).reshape(L // 16, 16).T
                mask_a[k, 16 * g:16 * g + 16, :L] = \
                    (sv & 1).astype(np.uint8)[None, :]
                mask_a[k, 64 + 16 * g:64 + 16 * g + 16, :L] = \
                    (tv & 1).astype(np.uint8)[None, :]
            e += 4 * L
        in_maps.append({
            "x0t": x0t,
            "wbig": wbig,
            "msel": msel,
            "x1": x_1[lo:lo + E_CORE],
            "idx": idx_a,
            "mask": mask_a,
        })
    return in_maps


def kernel(x_0, adjacency_0, x_1, att_parameter, _trace=False):
    # materialize as numpy up front: slicing jax arrays here would trigger
    # device jit compiles of generic XLA ops, which this toolchain rejects
    x_0 = np.asarray(x_0, dtype=np.float32)
    adjacency_0 = np.asarray(adjacency_0)
    x_1 = np.asarray(x_1, dtype=np.float32)
    att_parameter = np.asarray(att_parameter, dtype=np.float32)
    nc = _build_program()
    in_maps = _prep_inputs(x_0, adjacency_0, x_1, att_parameter)
    res = run_bass_kernel_spmd(nc, in_maps, list(range(N_CORES)), trace=_trace)
    out = np.concatenate([res.results[i]["out"] for i in range(N_CORES)], axis=0)
    kernel.last_exec_time_ns = res.exec_time_ns
    return out


# revision 10
# speedup vs baseline: 1.0027x; 1.0023x over previous
"""MultiHeadLiftLayer Trainium2 kernel.

reference:
    edge_signal = relu(x_0[src] @ W[:C] + x_0[tgt] @ W[C:])   # [E, 8]
    out = concat([edge_signal, x_1], axis=1)                   # [E, 72]

Strategy (8 NeuronCores, edges sharded):
  - Precompute per-node projections P_src = x_0 @ W[:C], P_tgt = x_0 @ W[C:]
    (each [N, 8]) on the tensor engine, stored as an f16 pair-packed table in
    SBUF: partition p holds one head-column (heads replicated; partitions
    0-63 = src heads, 64-127 = tgt heads), two consecutive nodes packed per
    u32 element -> num_elems 25000 fits ap_gather's int16-delta constraint.
  - Per 8192-edge call: GPSIMD ap_gather fetches the node pair for each
    edge (groups 0-3 use src indices of chunks 0-3, groups 4-7 tgt indices),
    DVE selects the even/odd f16 by node parity (host-provided u8 mask), and
    one PE matmul per 128-edge block against a fixed 0/1 selector sums the
    src/tgt lanes per head while landing directly in [edge, head] PSUM
    orientation. Rows are assembled in SBUF (p-major: partition p owns 64
    consecutive edges, so x_1 loads and output stores are one contiguous
    16-18KB DRAM run per partition) and stored with relu fused into the
    PSUM->SBUF copies.

    Measured on trn2: ap_gather runs ~28ns/idx (SBUF round-trip bound in the
    ucode, 4 idx per pipelined-depth-1 request); with 2 idx/edge spread over
    8 Q7 cores that is ~7ns/edge = ~545us for 78125 edges/core, which bounds
    the kernel; all DMA/PE/DVE/ACT work hides underneath it.
"""
import sys

sys.path.insert(0, "/opt/trn_rl_repo")

import numpy as np
import concourse.bass as bass
import concourse.tile as tile
from concourse import bacc, library_config, mybir
from concourse.bass_utils import run_bass_kernel_spmd

NUM_NODES = 50000
IN_CH0 = 128
HEADS = 8
NUM_EDGES = 625000
IN_CH1 = 64
OUT_CH = HEADS + IN_CH1  # 72

N_CORES = 8
E_CORE = NUM_EDGES // N_CORES  # 78125
L_MAIN = 2048                  # gather indices per call (per 16-partition group)
N_MAIN = 9                     # main calls: 9 * 4 * 2048 = 73728 edges
L_TAIL = 1152                  # tail call: 4 * 1152 = 4608 slots, 4397 valid
CALL_LS = [L_MAIN] * N_MAIN + [L_TAIL]
NPAIR = NUM_NODES // 2         # 25000 u32 elements per table column
NT = 2000                      # node-tile for the projection matmul
PCHUNK = 500                   # psum free-dim chunk

_cache = {}


def _build_program():
    if "nc" in _cache:
        return _cache["nc"]
    nc = bacc.Bacc("TRN2", target_bir_lowering=False, debug=False,
                   num_devices=N_CORES)
    f32, f16, i16 = mybir.dt.float32, mybir.dt.float16, mybir.dt.int16
    u8 = mybir.dt.uint8

    x0t = nc.dram_tensor("x0t", [IN_CH0, NUM_NODES], f16, kind="ExternalInput").ap()
    wbig = nc.dram_tensor("wbig", [IN_CH0, 128], f32, kind="ExternalInput").ap()
    x1 = nc.dram_tensor("x1", [E_CORE, IN_CH1], f32, kind="ExternalInput").ap()
    idx_in = nc.dram_tensor("idx", [len(CALL_LS), 128, L_MAIN // 16], i16,
                            kind="ExternalInput").ap()
    msel_in = nc.dram_tensor("msel", [128, 32], f32, kind="ExternalInput").ap()
    mask_in = nc.dram_tensor("mask", [len(CALL_LS), 128, L_MAIN], u8,
                             kind="ExternalInput").ap()
    out = nc.dram_tensor("out", [E_CORE, OUT_CH], f32, kind="ExternalOutput").ap()

    with tile.TileContext(nc) as tc:
        with tc.tile_pool(name="tab", bufs=1) as tab_pool, \
             tc.tile_pool(name="const", bufs=1) as const_pool:
            tab = tab_pool.tile([128, NPAIR], f32)       # f16 pair-packed view
            tab_f16 = tab[:].bitcast(f16)                # [128, 50000]
            msel32 = const_pool.tile([128, 32], f32)
            nc.sync.dma_start(msel32[:], msel_in[:])
            msel = const_pool.tile([128, 32], f16)
            nc.vector.tensor_copy(msel[:], msel32[:])
            # hoist the ~58us GPSIMD ucode-library swap for ap_gather to the
            # start so it overlaps phase 1 (GPSIMD is idle until the first
            # gather; without this the auto-inserted reload stalls the kernel
            # between the last table write and the first gather call)
            nc.gpsimd.load_library(library_config.ap_gather)

            # ---- phase 1: build the projection table ----
            with tc.tile_pool(name="p1", bufs=5) as p1_pool, \
                 tc.tile_pool(name="p1w", bufs=1) as p1w_pool, \
                 tc.tile_pool(name="p1ps", bufs=8, space="PSUM") as p1ps:
                wb32 = p1w_pool.tile([128, 128], f32)
                nc.sync.dma_start(wb32[:], wbig[:])
                wb16 = p1w_pool.tile([128, 128], f16)
                nc.vector.tensor_copy(wb16[:], wb32[:])
                ci = 0
                for t in range(NUM_NODES // NT):
                    xt = p1_pool.tile([128, NT], f16, tag="xt")
                    nc.sync.dma_start(xt[:], x0t[:, t * NT:(t + 1) * NT])
                    for c in range(NT // PCHUNK):
                        ps = p1ps.tile([128, PCHUNK], f32)
                        nc.tensor.matmul(ps[:], lhsT=wb16[:],
                                         rhs=xt[:, c * PCHUNK:(c + 1) * PCHUNK],
                                         start=True, stop=True)
                        n0 = t * NT + c * PCHUNK
                        dst = tab_f16[:, n0:n0 + PCHUNK]
                        if ci % 2 == 0:
                            nc.vector.tensor_copy(dst, ps[:])
                        else:
                            nc.scalar.copy(dst, ps[:])
                        ci += 1

            # ---- phase 2: gather / combine / emit ----
            with tc.tile_pool(name="io", bufs=3) as io_pool, \
                 tc.tile_pool(name="idxp", bufs=1) as idx_pool, \
                 tc.tile_pool(name="mega", bufs=2) as mega_pool, \
                 tc.tile_pool(name="p2ps", bufs=2, space="PSUM") as p2ps:
                its = []
                for k, L in enumerate(CALL_LS):
                    it = idx_pool.tile([128, L_MAIN // 16], i16, tag=f"it{k}")
                    nc.sync.dma_start(it[:, :L // 16], idx_in[k, :, :L // 16])
                    its.append(it)
                e_base = 0
                for k, L in enumerate(CALL_LS):
                    nseg = 4 * L // 128
                    it = its[k]
                    mk = io_pool.tile([128, L_MAIN], u8, tag="mk")
                    nc.sync.dma_start(mk[:, :L], mask_in[k, :, :L])

                    ot = io_pool.tile([128, L_MAIN], f32, tag="ot")
                    nc.gpsimd.ap_gather(out_ap=ot[:, :L], in_ap=tab[:],
                                        idxs_ap=it[:, :L // 16], channels=128,
                                        num_elems=NPAIR, d=1, num_idxs=L)
                    pair = ot[:, :L].bitcast(f16).rearrange(
                        "p (l two) -> p l two", two=2)
                    sel = io_pool.tile([128, L_MAIN], f16, tag="sel")
                    nc.vector.tensor_copy(sel[:, :L], pair[:, :, 0])
                    nc.vector.copy_predicated(sel[:, :L], mk[:, :L], pair[:, :, 1])

                    # per 128-edge block: one PE matmul sums the src lane
                    # and tgt lane per head (fixed 0/1 selector as the moving
                    # operand) and lands directly in [edge, head] orientation:
                    # psum[e, 8g+h] = sel[16g+h, e] + sel[64+16g+h, e]
                    nb = L // 128
                    ps2 = p2ps.tile([128, 512], f32)
                    for b in range(nb):
                        nc.tensor.matmul(ps2[:, 32 * b:32 * b + 32],
                                         lhsT=sel[:, 128 * b:128 * (b + 1)],
                                         rhs=msel[:], start=True, stop=True)

                    mega = mega_pool.tile([128, 64, OUT_CH], f32)
                    # relu fused into the PSUM->SBUF copies; chunk g block b
                    # sits at psum cols [32b + 8g : +8], destination seg g*nb+b
                    psv = ps2[:, :32 * nb].rearrange("p (s h) -> p s h", h=32)
                    for g in range(4):
                        nc.scalar.activation(
                            mega[:, g * nb:(g + 1) * nb, :HEADS],
                            psv[:, :, 8 * g:8 * g + 8],
                            mybir.ActivationFunctionType.Relu)

                    if k < N_MAIN:
                        # p-major: partition p holds edges [e_base+64p, +64),
                        # giving one contiguous 16-18KB DRAM run per partition
                        v = slice(e_base, e_base + 4 * L)
                        nc.sync.dma_start(
                            mega[:, :, HEADS:],
                            x1[v].rearrange("(p s) c -> p s c", s=64))
                        nc.scalar.dma_start(
                            out[v].rearrange("(p s) c -> p s c", s=64),
                            mega[:])
                    else:
                        # tail: seg-major with partial coverage
                        n_edges = min(E_CORE - e_base, 4 * L)
                        full_seg = n_edges // 128
                        rem = n_edges - full_seg * 128
                        if full_seg:
                            v = slice(e_base, e_base + full_seg * 128)
                            nc.sync.dma_start(
                                mega[:, :full_seg, HEADS:],
                                x1[v].rearrange("(s p) c -> p s c", p=128))
                            nc.scalar.dma_start(
                                out[v].rearrange("(s p) c -> p s c", p=128),
                                mega[:, :full_seg, :])
                        if rem:
                            v = slice(e_base + full_seg * 128, e_base + n_edges)
                            nc.sync.dma_start(mega[:rem, full_seg, HEADS:], x1[v])
                            nc.scalar.dma_start(out[v], mega[:rem, full_seg, :])
                    e_base += 4 * L

    nc.compile()
    _cache["nc"] = nc
    return nc


def _prep_inputs(x_0, adjacency_0, x_1, att_parameter):
    x0t = np.ascontiguousarray(np.asarray(x_0).T).astype(np.float16)
    wbig = np.empty((IN_CH0, 128), np.float32)
    for p in range(128):
        half = IN_CH0 * (p >= 64)
        wbig[:, p] = att_parameter[half:half + IN_CH0, p % 8]

    msel = np.zeros((128, 32), np.float32)
    for g in range(4):
        for h in range(8):
            msel[16 * g + h, 8 * g + h] = 1.0
            msel[64 + 16 * g + h, 8 * g + h] = 1.0

    src_all = np.asarray(adjacency_0[0]).astype(np.int64)
    tgt_all = np.asarray(adjacency_0[1]).astype(np.int64)
    x_1 = np.asarray(x_1, dtype=np.float32)

    in_maps = []
    for core in range(N_CORES):
        lo = core * E_CORE
        src = src_all[lo:lo + E_CORE]
        tgt = tgt_all[lo:lo + E_CORE]
        idx_a = np.zeros((len(CALL_LS), 128, L_MAIN // 16), np.int16)
        mask_a = np.zeros((len(CALL_LS), 128, L_MAIN), np.uint8)
        e = 0
        pos = np.arange(L_MAIN)
        pmaj = 64 * (pos % 128) + (pos // 128)  # i = 128b+p -> 64p + b
        for k, L in enumerate(CALL_LS):
            for g in range(4):
                if k < N_MAIN:
                    eoff = e + pmaj + 16 * g
                    sv = src[eoff]
                    tv = tgt[eoff]
                else:
                    c0 = e + g * L
                    sv = src[c0:c0 + L]
                    tv = tgt[c0:c0 + L]
                    if len(sv) < L:  # tail padding
                        sv = np.concatenate([sv, np.zeros(L - len(sv), np.int64)])
                        tv = np.concatenate([tv, np.zeros(L - len(tv), np.int64)])
                # wrapped: idxs[p, s] = v[16 s + p]
                idx_a[k, 16 * g:16 * g + 16, :L // 16] = \
                    (sv >> 1).astype(np.int16).reshape(L // 16, 16).T
                idx_a[k, 64 + 16 * g:64 + 16 * g + 16, :L // 16] = \
                    (tv >> 1).astype(np.int16).reshape(L // 16, 16).T
                mask_a[k, 16 * g:16 * g + 16, :L] = \
                    (sv & 1).astype(np.uint8)[None, :]
                mask_a[k, 64 + 16 * g:64 + 16 * g + 16, :L] = \
                    (tv & 1).astype(np.uint8)[None, :]
            e += 4 * L
        in_maps.append({
            "x0t": x0t,
            "wbig": wbig,
            "msel": msel,
            "x1": x_1[lo:lo + E_CORE],
            "idx": idx_a,
            "mask": mask_a,
        })
    return in_maps


def kernel(x_0, adjacency_0, x_1, att_parameter, _trace=False):
    # materialize as numpy up front: slicing jax arrays here would trigger
    # device jit compiles of generic XLA ops, which this toolchain rejects
    x_0 = np.asarray(x_0, dtype=np.float32)
    adjacency_0 = np.asarray(adjacency_0)
    x_1 = np.asarray(x_1, dtype=np.float32)
    att_parameter = np.asarray(att_parameter, dtype=np.float32)
    nc = _build_program()
    in_maps = _prep_inputs(x_0, adjacency_0, x_1, att_parameter)
    res = run_bass_kernel_spmd(nc, in_maps, list(range(N_CORES)), trace=_trace)
    out = np.concatenate([res.results[i]["out"] for i in range(N_CORES)], axis=0)
    kernel.last_exec_time_ns = res.exec_time_ns
    return out


# revision 11
# speedup vs baseline: 1.0041x; 1.0014x over previous
"""MultiHeadLiftLayer Trainium2 kernel.

reference:
    edge_signal = relu(x_0[src] @ W[:C] + x_0[tgt] @ W[C:])   # [E, 8]
    out = concat([edge_signal, x_1], axis=1)                   # [E, 72]

Strategy (8 NeuronCores, edges sharded):
  - Precompute per-node projections P_src = x_0 @ W[:C], P_tgt = x_0 @ W[C:]
    (each [N, 8]) on the tensor engine, stored as an f16 pair-packed table in
    SBUF: partition p holds one head-column (heads replicated; partitions
    0-63 = src heads, 64-127 = tgt heads), two consecutive nodes packed per
    u32 element -> num_elems 25000 fits ap_gather's int16-delta constraint.
  - Per 8192-edge call: GPSIMD ap_gather fetches the node pair for each
    edge (groups 0-3 use src indices of chunks 0-3, groups 4-7 tgt indices),
    DVE selects the even/odd f16 by node parity (host-provided u8 mask), and
    one PE matmul per 128-edge block against a fixed 0/1 selector sums the
    src/tgt lanes per head while landing directly in [edge, head] PSUM
    orientation. Rows are assembled in SBUF (p-major: partition p owns 64
    consecutive edges, so x_1 loads and output stores are one contiguous
    16-18KB DRAM run per partition) and stored with relu fused into the
    PSUM->SBUF copies.

    Measured on trn2: ap_gather runs ~28ns/idx (SBUF round-trip bound in the
    ucode, 4 idx per pipelined-depth-1 request); with 2 idx/edge spread over
    8 Q7 cores that is ~7ns/edge = ~545us for 78125 edges/core, which bounds
    the kernel; all DMA/PE/DVE/ACT work hides underneath it.
"""
import sys

sys.path.insert(0, "/opt/trn_rl_repo")

import numpy as np
import concourse.bass as bass
import concourse.tile as tile
from concourse import bacc, library_config, mybir
from concourse.bass_utils import run_bass_kernel_spmd

NUM_NODES = 50000
IN_CH0 = 128
HEADS = 8
NUM_EDGES = 625000
IN_CH1 = 64
OUT_CH = HEADS + IN_CH1  # 72

N_CORES = 8
E_CORE = NUM_EDGES // N_CORES  # 78125
L_MAIN = 2048                  # gather indices per call (per 16-partition group)
N_MAIN = 9                     # main calls: 9 * 4 * 2048 = 73728 edges
L_TAIL = 1152                  # tail call: 4 * 1152 = 4608 slots, 4397 valid
CALL_LS = [L_MAIN] * N_MAIN + [L_TAIL]
NPAIR = NUM_NODES // 2         # 25000 u32 elements per table column
NT = 2000                      # node-tile for the projection matmul
PCHUNK = 500                   # psum free-dim chunk

_cache = {}


def _build_program():
    if "nc" in _cache:
        return _cache["nc"]
    nc = bacc.Bacc("TRN2", target_bir_lowering=False, debug=False,
                   num_devices=N_CORES)
    f32, f16, i16 = mybir.dt.float32, mybir.dt.float16, mybir.dt.int16
    u8 = mybir.dt.uint8

    x0t = nc.dram_tensor("x0t", [IN_CH0, NUM_NODES], f16, kind="ExternalInput").ap()
    wbig = nc.dram_tensor("wbig", [IN_CH0, 128], f32, kind="ExternalInput").ap()
    x1 = nc.dram_tensor("x1", [E_CORE, IN_CH1], f32, kind="ExternalInput").ap()
    idx_in = nc.dram_tensor("idx", [len(CALL_LS), 128, L_MAIN // 16], i16,
                            kind="ExternalInput").ap()
    msel_in = nc.dram_tensor("msel", [128, 32], f32, kind="ExternalInput").ap()
    mask_in = nc.dram_tensor("mask", [len(CALL_LS), 128, L_MAIN], u8,
                             kind="ExternalInput").ap()
    out = nc.dram_tensor("out", [E_CORE, OUT_CH], f32, kind="ExternalOutput").ap()

    with tile.TileContext(nc) as tc:
        with tc.tile_pool(name="tab", bufs=1) as tab_pool, \
             tc.tile_pool(name="const", bufs=1) as const_pool:
            tab = tab_pool.tile([128, NPAIR], f32)       # f16 pair-packed view
            tab_f16 = tab[:].bitcast(f16)                # [128, 50000]
            msel32 = const_pool.tile([128, 32], f32)
            nc.sync.dma_start(msel32[:], msel_in[:])
            msel = const_pool.tile([128, 32], f16)
            nc.vector.tensor_copy(msel[:], msel32[:])
            # hoist the ~58us GPSIMD ucode-library swap for ap_gather to the
            # start so it overlaps phase 1 (GPSIMD is idle until the first
            # gather; without this the auto-inserted reload stalls the kernel
            # between the last table write and the first gather call)
            nc.gpsimd.load_library(library_config.ap_gather)

            # ---- phase 1: build the projection table ----
            with tc.tile_pool(name="p1", bufs=5) as p1_pool, \
                 tc.tile_pool(name="p1w", bufs=1) as p1w_pool, \
                 tc.tile_pool(name="p1ps", bufs=8, space="PSUM") as p1ps:
                wb32 = p1w_pool.tile([128, 128], f32)
                nc.sync.dma_start(wb32[:], wbig[:])
                wb16 = p1w_pool.tile([128, 128], f16)
                nc.vector.tensor_copy(wb16[:], wb32[:])
                ci = 0
                for t in range(NUM_NODES // NT):
                    xt = p1_pool.tile([128, NT], f16, tag="xt")
                    nc.sync.dma_start(xt[:], x0t[:, t * NT:(t + 1) * NT])
                    for c in range(NT // PCHUNK):
                        ps = p1ps.tile([128, PCHUNK], f32)
                        nc.tensor.matmul(ps[:], lhsT=wb16[:],
                                         rhs=xt[:, c * PCHUNK:(c + 1) * PCHUNK],
                                         start=True, stop=True)
                        n0 = t * NT + c * PCHUNK
                        dst = tab_f16[:, n0:n0 + PCHUNK]
                        if ci % 2 == 0:
                            nc.vector.tensor_copy(dst, ps[:])
                        else:
                            nc.scalar.copy(dst, ps[:])
                        ci += 1

            # ---- phase 2: gather / combine / emit ----
            with tc.tile_pool(name="io", bufs=3) as io_pool, \
                 tc.tile_pool(name="idxp", bufs=1) as idx_pool, \
                 tc.tile_pool(name="mega", bufs=2) as mega_pool, \
                 tc.tile_pool(name="p2ps", bufs=2, space="PSUM") as p2ps:
                its = []
                for k, L in enumerate(CALL_LS):
                    it = idx_pool.tile([128, L_MAIN // 16], i16, tag=f"it{k}")
                    nc.scalar.dma_start(it[:, :L // 16], idx_in[k, :, :L // 16])
                    its.append(it)
                e_base = 0
                for k, L in enumerate(CALL_LS):
                    nseg = 4 * L // 128
                    it = its[k]
                    mk = io_pool.tile([128, L_MAIN], u8, tag="mk")
                    nc.scalar.dma_start(mk[:, :L], mask_in[k, :, :L])

                    ot = io_pool.tile([128, L_MAIN], f32, tag="ot")
                    nc.gpsimd.ap_gather(out_ap=ot[:, :L], in_ap=tab[:],
                                        idxs_ap=it[:, :L // 16], channels=128,
                                        num_elems=NPAIR, d=1, num_idxs=L)
                    pair = ot[:, :L].bitcast(f16).rearrange(
                        "p (l two) -> p l two", two=2)
                    sel = io_pool.tile([128, L_MAIN], f16, tag="sel")
                    nc.vector.tensor_copy(sel[:, :L], pair[:, :, 0])
                    nc.vector.copy_predicated(sel[:, :L], mk[:, :L], pair[:, :, 1])

                    # per 128-edge block: one PE matmul sums the src lane
                    # and tgt lane per head (fixed 0/1 selector as the moving
                    # operand) and lands directly in [edge, head] orientation:
                    # psum[e, 8g+h] = sel[16g+h, e] + sel[64+16g+h, e]
                    nb = L // 128
                    ps2 = p2ps.tile([128, 512], f32)
                    for b in range(nb):
                        nc.tensor.matmul(ps2[:, 32 * b:32 * b + 32],
                                         lhsT=sel[:, 128 * b:128 * (b + 1)],
                                         rhs=msel[:], start=True, stop=True)

                    mega = mega_pool.tile([128, 64, OUT_CH], f32)
                    # relu fused into the PSUM->SBUF copies; chunk g block b
                    # sits at psum cols [32b + 8g : +8], destination seg g*nb+b
                    psv = ps2[:, :32 * nb].rearrange("p (s h) -> p s h", h=32)
                    for g in range(4):
                        nc.scalar.activation(
                            mega[:, g * nb:(g + 1) * nb, :HEADS],
                            psv[:, :, 8 * g:8 * g + 8],
                            mybir.ActivationFunctionType.Relu)

                    if k < N_MAIN:
                        # p-major: partition p holds edges [e_base+64p, +64),
                        # giving one contiguous 16-18KB DRAM run per partition
                        v = slice(e_base, e_base + 4 * L)
                        nc.sync.dma_start(
                            mega[:, :, HEADS:],
                            x1[v].rearrange("(p s) c -> p s c", s=64))
                        nc.scalar.dma_start(
                            out[v].rearrange("(p s) c -> p s c", s=64),
                            mega[:])
                    else:
                        # tail: seg-major with partial coverage
                        n_edges = min(E_CORE - e_base, 4 * L)
                        full_seg = n_edges // 128
                        rem = n_edges - full_seg * 128
                        if full_seg:
                            v = slice(e_base, e_base + full_seg * 128)
                            nc.sync.dma_start(
                                mega[:, :full_seg, HEADS:],
                                x1[v].rearrange("(s p) c -> p s c", p=128))
                            nc.scalar.dma_start(
                                out[v].rearrange("(s p) c -> p s c", p=128),
                                mega[:, :full_seg, :])
                        if rem:
                            v = slice(e_base + full_seg * 128, e_base + n_edges)
                            nc.sync.dma_start(mega[:rem, full_seg, HEADS:], x1[v])
                            nc.scalar.dma_start(out[v], mega[:rem, full_seg, :])
                    e_base += 4 * L

    nc.compile()
    _cache["nc"] = nc
    return nc


def _prep_inputs(x_0, adjacency_0, x_1, att_parameter):
    x0t = np.ascontiguousarray(np.asarray(x_0).T).astype(np.float16)
    wbig = np.empty((IN_CH0, 128), np.float32)
    for p in range(128):
        half = IN_CH0 * (p >= 64)
        wbig[:, p] = att_parameter[half:half + IN_CH0, p % 8]

    msel = np.zeros((128, 32), np.float32)
    for g in range(4):
        for h in range(8):
            msel[16 * g + h, 8 * g + h] = 1.0
            msel[64 + 16 * g + h, 8 * g + h] = 1.0

    src_all = np.asarray(adjacency_0[0]).astype(np.int64)
    tgt_all = np.asarray(adjacency_0[1]).astype(np.int64)
    x_1 = np.asarray(x_1, dtype=np.float32)

    in_maps = []
    for core in range(N_CORES):
        lo = core * E_CORE
        src = src_all[lo:lo + E_CORE]
        tgt = tgt_all[lo:lo + E_CORE]
        idx_a = np.zeros((len(CALL_LS), 128, L_MAIN // 16), np.int16)
        mask_a = np.zeros((len(CALL_LS), 128, L_MAIN), np.uint8)
        e = 0
        pos = np.arange(L_MAIN)
        pmaj = 64 * (pos % 128) + (pos // 128)  # i = 128b+p -> 64p + b
        for k, L in enumerate(CALL_LS):
            for g in range(4):
                if k < N_MAIN:
                    eoff = e + pmaj + 16 * g
                    sv = src[eoff]
                    tv = tgt[eoff]
                else:
                    c0 = e + g * L
                    sv = src[c0:c0 + L]
                    tv = tgt[c0:c0 + L]
                    if len(sv) < L:  # tail padding
                        sv = np.concatenate([sv, np.zeros(L - len(sv), np.int64)])
                        tv = np.concatenate([tv, np.zeros(L - len(tv), np.int64)])
                # wrapped: idxs[p, s] = v[16 s + p]
                idx_a[k, 16 * g:16 * g + 16, :L // 16] = \
                    (sv >> 1).astype(np.int16).reshape(L // 16, 16).T
                idx_a[k, 64 + 16 * g:64 + 16 * g + 16, :L // 16] = \
                    (tv >> 1).astype(np.int16).reshape(L // 16, 16).T
                mask_a[k, 16 * g:16 * g + 16, :L] = \
                    (sv & 1).astype(np.uint8)[None, :]
                mask_a[k, 64 + 16 * g:64 + 16 * g + 16, :L] = \
                    (tv & 1).astype(np.uint8)[None, :]
            e += 4 * L
        in_maps.append({
            "x0t": x0t,
            "wbig": wbig,
            "msel": msel,
            "x1": x_1[lo:lo + E_CORE],
            "idx": idx_a,
            "mask": mask_a,
        })
    return in_maps


def kernel(x_0, adjacency_0, x_1, att_parameter, _trace=False):
    # materialize as numpy up front: slicing jax arrays here would trigger
    # device jit compiles of generic XLA ops, which this toolchain rejects
    x_0 = np.asarray(x_0, dtype=np.float32)
    adjacency_0 = np.asarray(adjacency_0)
    x_1 = np.asarray(x_1, dtype=np.float32)
    att_parameter = np.asarray(att_parameter, dtype=np.float32)
    nc = _build_program()
    in_maps = _prep_inputs(x_0, adjacency_0, x_1, att_parameter)
    res = run_bass_kernel_spmd(nc, in_maps, list(range(N_CORES)), trace=_trace)
    out = np.concatenate([res.results[i]["out"] for i in range(N_CORES)], axis=0)
    kernel.last_exec_time_ns = res.exec_time_ns
    return out
